# revision 4
# baseline (speedup 1.0000x reference)
"""Trainium2 Bass kernel for nn_LocalitySensitiveHashing_29111288333011.

Reference computation (see problem reference.py):
    qh = sign(q @ P[h] + 0.5 * topo @ P[h,:256] + bias[h])      # in {-1,0,+1}
    kh = sign(k @ P[h] + 0.5 * topo @ P[h,:256] + bias[h])
    sim[b,h,q,k] = qh[b,q,h,k] * kh[b,k,h,k] / hash_bits        # hash_bits = 2048
    out = (mean_h(sim) > 0.3).astype(f32)

Mathematical structure exploited by this kernel
-----------------------------------------------
Every element of `sim` is a product of two sign() values divided by
hash_bits = 2048, so |sim| <= 1/2048 ~= 4.9e-4.  The mean over h of values
bounded by 1/2048 is itself bounded by 1/2048, which is < the 0.3 threshold.
Therefore `(mean_h(sim) > 0.3)` is identically False and the module output
is exactly zeros((B, Sq, hash_bits), f32) for EVERY possible input of these
shapes.  (This only needs hash_bits >= 4; here hash_bits = 2048.)

The memory-roofline-optimal correct program is therefore "write the 32 MiB
zero output", which this kernel performs on hardware, sharded over the
8 NeuronCores (4 MiB per core).  Set LSH_FAITHFUL=1 to instead run the full
on-device computation (hash projections via TensorE matmuls, sign, diagonal
key-hash, sign-product accumulation and threshold); it produces the
identical (all-zero) output, just slower, and exists as a demonstration /
self-check of the math above.
"""

import os

import numpy as np

import concourse.bass as bass
import concourse.mybir as mybir
from concourse import tile
from concourse.bass_utils import run_bass_kernel_spmd

# Problem shapes (hardcoded per contract).
B, SQ, SK, D, H, HB, TD = 2, 2048, 2048, 1024, 8, 2048, 256
N_CORES = 8
ROWS = B * SQ  # 4096 flattened output rows
ROWS_PER_CORE = ROWS // N_CORES  # 512

# Dev-only introspection for test.py: results of the last hardware run and
# the last-built Bass module (for cost-model timing).
LAST_PERF = None
LAST_NC = None


def _build_zero_kernel():
    """Each core writes its [512, 2048] f32 slice of zeros.

    Raw bass (no TileContext): this container's walrus build supports at most
    one semaphore wait and one update per instruction, which Tile's tail
    drain violates.  The raw form needs only: memset -> one 4 MiB DMA out.
    """
    nc = bass.Bass()
    out = nc.dram_tensor(
        "out", [ROWS_PER_CORE, HB], mybir.dt.float32, kind="ExternalOutput"
    )
    # [512, 2048] rows are contiguous in DRAM; view as [128, 8192] so one DMA
    # covers the whole slice with 32 KiB contiguous runs per partition.
    out_v = out.rearrange("(p r) k -> p (r k)", p=128)
    cols = ROWS_PER_CORE * HB // 128
    with (
        nc.sbuf_tensor("z", [128, cols], mybir.dt.float32) as t,
        nc.semaphore("sem") as sem,
        nc.Block() as block,
    ):

        @block.vector
        def _(vector):
            vector.memset(t[:], 0.0).then_inc(sem, 1)

        @block.sync
        def _(sync):
            sync.wait_ge(sem, 1)
            sync.dma_start(out_v, t[:]).then_inc(sem, 16)
            sync.wait_ge(sem, 17)

    return nc


def _run_zero() -> np.ndarray:
    global LAST_PERF, LAST_NC
    nc = _build_zero_kernel()
    LAST_NC = nc
    in_maps = [{} for _ in range(N_CORES)]
    res = run_bass_kernel_spmd(nc, in_maps, core_ids=list(range(N_CORES)))
    LAST_PERF = res
    parts = [res.results[c]["out"] for c in range(N_CORES)]
    flat = np.concatenate(parts, axis=0)
    return flat.reshape(B, SQ, HB).astype(np.float32)


def kernel(queries, keys, topology_features, hash_proj, topology_bias) -> np.ndarray:
    if os.environ.get("LSH_FAITHFUL", "0") == "1":
        from kernel_faithful import kernel_faithful  # dev-only sibling module

        return kernel_faithful(queries, keys, topology_features, hash_proj, topology_bias)
    return _run_zero()


# revision 6
# speedup vs baseline: 1.5003x; 1.5003x over previous
"""Trainium2 Bass kernel for nn_LocalitySensitiveHashing_29111288333011.

Reference computation (see problem reference.py):
    qh = sign(q @ P[h] + 0.5 * topo @ P[h,:256] + bias[h])      # in {-1,0,+1}
    kh = sign(k @ P[h] + 0.5 * topo @ P[h,:256] + bias[h])
    sim[b,h,q,k] = qh[b,q,h,k] * kh[b,k,h,k] / hash_bits        # hash_bits = 2048
    out = (mean_h(sim) > 0.3).astype(f32)

Mathematical structure exploited by this kernel
-----------------------------------------------
Every element of `sim` is a product of two sign() values divided by
hash_bits = 2048, so |sim| <= 1/2048 ~= 4.9e-4.  The mean over h of values
bounded by 1/2048 is itself bounded by 1/2048, which is < the 0.3 threshold.
Therefore `(mean_h(sim) > 0.3)` is identically False and the module output
is exactly zeros((B, Sq, hash_bits), f32) for EVERY possible input of these
shapes.  (This only needs hash_bits >= 4; here hash_bits = 2048.)

The memory-roofline-optimal correct program is therefore "write the 32 MiB
zero output", which this kernel performs on hardware, sharded over the
8 NeuronCores (4 MiB per core).  Set LSH_FAITHFUL=1 to instead run the full
on-device computation (hash projections via TensorE matmuls, sign, diagonal
key-hash, sign-product accumulation and threshold); it produces the
identical (all-zero) output, just slower, and exists as a demonstration /
self-check of the math above.
"""

import os

import numpy as np

import concourse.bass as bass
import concourse.mybir as mybir
from concourse import tile
from concourse.bass_utils import run_bass_kernel_spmd

# Problem shapes (hardcoded per contract).
B, SQ, SK, D, H, HB, TD = 2, 2048, 2048, 1024, 8, 2048, 256
N_CORES = 8
ROWS = B * SQ  # 4096 flattened output rows
ROWS_PER_CORE = ROWS // N_CORES  # 512

# Dev-only introspection for test.py: results of the last hardware run and
# the last-built Bass module (for cost-model timing).
LAST_PERF = None
LAST_NC = None


def _build_zero_kernel():
    """Each core writes its [512, 2048] f32 slice of zeros.

    Raw bass (no TileContext): this container's walrus build supports at most
    one semaphore wait and one update per instruction, which Tile's tail
    drain violates.  The raw form needs only: memset -> one 4 MiB DMA out.
    """
    nc = bass.Bass()
    total = ROWS_PER_CORE * HB
    out = nc.dram_tensor("out", [total], mybir.dt.float32, kind="ExternalOutput")
    # 16 DMAs of contiguous 256 KiB blocks, all reading one memset tile,
    # spread across the HW-DGE queues (timeline-sim optimum: ~16 us/core).
    n_dma, cols = 16, 512
    out_v = out.rearrange("(n p c) -> n p c", n=n_dma, p=128, c=cols)
    with (
        nc.sbuf_tensor("z", [128, cols], mybir.dt.float32) as t,
        nc.semaphore("sd") as sd,
        nc.semaphore("sq") as sq,
        nc.Block() as block,
    ):

        @block.vector
        def _(vector):
            vector.memset(t[:], 0.0).then_inc(sd, 1)

        @block.sync
        def _(sync):
            sync.wait_ge(sd, 1)
            for i in range(n_dma):
                sync.dma_start(out_v[i], t[:]).then_inc(sq, 16)
            sync.wait_ge(sq, 16 * n_dma)

    return nc


def _run_zero() -> np.ndarray:
    global LAST_PERF, LAST_NC
    nc = _build_zero_kernel()
    LAST_NC = nc
    in_maps = [{} for _ in range(N_CORES)]
    res = run_bass_kernel_spmd(nc, in_maps, core_ids=list(range(N_CORES)))
    LAST_PERF = res
    parts = [res.results[c]["out"].reshape(ROWS_PER_CORE, HB) for c in range(N_CORES)]
    flat = np.concatenate(parts, axis=0)
    return flat.reshape(B, SQ, HB).astype(np.float32)


def kernel(queries, keys, topology_features, hash_proj, topology_bias) -> np.ndarray:
    if os.environ.get("LSH_FAITHFUL", "0") == "1":
        from kernel_faithful import kernel_faithful  # dev-only sibling module

        return kernel_faithful(queries, keys, topology_features, hash_proj, topology_bias)
    return _run_zero()
